# revision 10
# baseline (speedup 1.0000x reference)
"""Trainium2 Bass kernel for nn_AddInterpolant (dense MLP + JVP interpolant).

Data-parallel over 8 NeuronCores: batch 65536 is split into 8 shards of
8192 rows; the small MLP weights are replicated.  Per core the kernel
computes, for z = concat(x0, x1, t):

    fnn    = W4.(relu(W3.(relu(W2.(relu(W1.z + b1)) + b2)) + b3)) + b4
    dt_fnn = d fnn / dt   (forward-mode JVP with one-hot tangent on t)
    xt     = (1-t) x0 + t x1 + t (1-t) fnn
    dt_xt  = x1 - x0 + (1-2t) fnn + t (1-t) dt_fnn

Layout: activations are kept transposed (features on SBUF partitions,
batch on the free axis) so every layer is a plain accumulation of
128x128 weight-block matmuls; inputs/outputs are transposed on the PE
via identity matmuls.  The t column of z is folded into a zero-padded
K=128 chunk so all matmuls are uniform.  Compute dtype is selectable:
float32r (full-rate fp32 path) or bfloat16.
"""

import os
import sys

for _p in ("/opt/trn_rl_repo",):
    if _p not in sys.path:
        sys.path.insert(0, _p)

import numpy as np

import concourse.mybir as mybir
import concourse.tile as tile
from concourse import bacc
from concourse.bass import ds
from concourse.bass_utils import run_bass_kernel_spmd
from concourse.masks import make_identity

P = 128
D = 256  # state dim
H = 1024  # hidden dim
B = 65536  # global batch
NCORES = 8
BL = B // NCORES  # rows per core
S = 512  # batch columns per stripe
NSTRIPES = BL // S
HC = H // P  # 8 hidden chunks
DC = D // P  # 2 state chunks

F32 = mybir.dt.float32
F32R = mybir.dt.float32r
BF16 = mybir.dt.bfloat16
RELU = mybir.ActivationFunctionType.Relu
IDENT = mybir.ActivationFunctionType.Identity
SIGN = mybir.ActivationFunctionType.Sign
GT = mybir.AluOpType.is_gt
MULT = mybir.AluOpType.mult
ADD = mybir.AluOpType.add
SUB = mybir.AluOpType.subtract
MAX = mybir.AluOpType.max

MODE = os.environ.get("KERNEL_MODE", "bf16")  # "bf16" | "f32r"

_nc_cache = {}


def _r(ap):
    return ap.bitcast(F32R)


def build(mode=None):
    mode = mode or MODE
    MMDT = BF16 if mode == "bf16" else F32R
    nc = bacc.Bacc(None)

    x0e = nc.declare_dram_parameter("x0", [BL, D], F32, isOutput=False)
    x1e = nc.declare_dram_parameter("x1", [BL, D], F32, isOutput=False)
    te = nc.declare_dram_parameter("t", [BL, 1], F32, isOutput=False)
    W1e = nc.declare_dram_parameter("W1", [2 * D + 1, H], F32, isOutput=False)
    b1e = nc.declare_dram_parameter("b1", [H], F32, isOutput=False)
    W2e = nc.declare_dram_parameter("W2", [H, H], F32, isOutput=False)
    b2e = nc.declare_dram_parameter("b2", [H], F32, isOutput=False)
    W3e = nc.declare_dram_parameter("W3", [H, H], F32, isOutput=False)
    b3e = nc.declare_dram_parameter("b3", [H], F32, isOutput=False)
    W4e = nc.declare_dram_parameter("W4", [H, D], F32, isOutput=False)
    b4e = nc.declare_dram_parameter("b4", [D], F32, isOutput=False)
    xte = nc.declare_dram_parameter("xt", [BL, D], F32, isOutput=True)
    dte = nc.declare_dram_parameter("dt_xt", [BL, D], F32, isOutput=True)

    with tile.TileContext(nc) as tc:
        with (
            tc.tile_pool(name="const", bufs=1) as cp,
            tc.tile_pool(name="z", bufs=1) as zp,
            tc.tile_pool(name="acts", bufs=1) as hp,
            tc.tile_pool(name="outs", bufs=1) as fp,
            tc.tile_pool(name="nat", bufs=2) as npl,
            tc.tile_pool(name="small", bufs=2) as sp,
            tc.tile_pool(name="mm", bufs=2, space="PSUM") as mmp,
            tc.tile_pool(name="tps", bufs=3, space="PSUM") as tpp,
        ):
            # ---- weights in compute dtype ----
            w1s = cp.tile([P, 4, H], MMDT)
            w2s = cp.tile([P, HC, H], MMDT)
            w3s = cp.tile([P, HC, H], MMDT)
            w4s = cp.tile([P, HC, D], MMDT)
            if mode == "f32r":
                # direct DMA via bitcast (verifier accepts f32r-typed DMA)
                nc.sync.dma_start(
                    w1s[:], _r(W1e[0 : 2 * D].rearrange("(o p) n -> p o n", p=P))
                )
                nc.sync.dma_start(w2s[:], _r(W2e.rearrange("(o p) n -> p o n", p=P)))
                nc.sync.dma_start(w3s[:], _r(W3e.rearrange("(o p) n -> p o n", p=P)))
                nc.sync.dma_start(w4s[:], _r(W4e.rearrange("(o p) n -> p o n", p=P)))
            else:
                wst = cp.tile([P, HC, H], F32, name="wstage")
                nc.sync.dma_start(
                    wst[:, 0:4, :], W1e[0 : 2 * D].rearrange("(o p) n -> p o n", p=P)
                )
                nc.vector.tensor_copy(w1s[:], wst[:, 0:4, :])
                nc.sync.dma_start(wst[:], W2e.rearrange("(o p) n -> p o n", p=P))
                nc.vector.tensor_copy(w2s[:], wst[:])
                nc.sync.dma_start(wst[:], W3e.rearrange("(o p) n -> p o n", p=P))
                nc.vector.tensor_copy(w3s[:], wst[:])
                nc.sync.dma_start(
                    wst[:, :, 0:D], W4e.rearrange("(o p) n -> p o n", p=P)
                )
                nc.vector.tensor_copy(w4s[:], wst[:, :, 0:D])
            w1rp = cp.tile([P, HC], F32)
            nc.sync.dma_start(w1rp[:], W1e[2 * D, :].rearrange("(o p) -> p o", p=P))
            b1p = cp.tile([P, HC], F32)
            nc.sync.dma_start(b1p[:], b1e.rearrange("(o p) -> p o", p=P))
            b2p = cp.tile([P, HC], F32)
            nc.sync.dma_start(b2p[:], b2e.rearrange("(o p) -> p o", p=P))
            b3p = cp.tile([P, HC], F32)
            nc.sync.dma_start(b3p[:], b3e.rearrange("(o p) -> p o", p=P))
            b4p = cp.tile([P, DC], F32)
            nc.sync.dma_start(b4p[:], b4e.rearrange("(o p) -> p o", p=P))
            ident = cp.tile([P, P], F32)
            make_identity(nc, ident)
            ident_m = cp.tile([P, P], MMDT)
            nc.vector.tensor_copy(ident_m[:], ident[:])

            # padded "t chunk": Z5 row0 = t (per stripe), rest 0; W15 row0 = W1[512]
            zstage = fp.tile([P, 4, D], F32, tag="dt_nat", name="zstage")
            nc.vector.memset(zstage[:], 0.0)
            z5 = cp.tile([P, S], MMDT)
            nc.vector.tensor_copy(
                z5[:], zstage[:, 0:2, :].rearrange("p a b -> p (a b)")
            )
            w15 = cp.tile([P, H], MMDT)
            nc.vector.tensor_copy(w15[:], zstage[:].rearrange("p a b -> p (a b)"))
            if mode == "f32r":
                nc.sync.dma_start(w15[0:1, :], _r(W1e[2 * D : 2 * D + 1, :]))
            else:
                w15st = sp.tile([1, H], F32, tag="w15st", bufs=1)
                nc.sync.dma_start(w15st[:], W1e[2 * D : 2 * D + 1, :])
                nc.vector.tensor_copy(w15[0:1, :], w15st[:])

            for s in range(NSTRIPES):
                row0 = s * S
                # ---- stripe inputs ----
                nat0 = npl.tile([P, 4, D], F32, tag="nat0")
                nat1 = npl.tile([P, 4, D], F32, tag="nat1")
                if mode == "f32r":
                    nc.sync.dma_start(
                        _r(nat0[:]),
                        _r(x0e[ds(row0, S), :].rearrange("(c p) f -> p c f", p=P)),
                    )
                    nc.sync.dma_start(
                        _r(nat1[:]),
                        _r(x1e[ds(row0, S), :].rearrange("(c p) f -> p c f", p=P)),
                    )
                    tsrc0, tsrc1 = nat0, nat1
                else:
                    nc.sync.dma_start(
                        nat0[:], x0e[ds(row0, S), :].rearrange("(c p) f -> p c f", p=P)
                    )
                    nc.sync.dma_start(
                        nat1[:], x1e[ds(row0, S), :].rearrange("(c p) f -> p c f", p=P)
                    )
                    # convert on idle GpSimd for 1-cyc/row bf16 transposes
                    natb0 = npl.tile([P, 4, D], BF16, tag="natb0")
                    nc.gpsimd.tensor_copy(natb0[:], nat0[:])
                    natb1 = npl.tile([P, 4, D], BF16, tag="natb1")
                    nc.gpsimd.tensor_copy(natb1[:], nat1[:])
                    tsrc0, tsrc1 = natb0, natb1
                if mode == "f32r":
                    nc.sync.dma_start(
                        z5[0:1, :], _r(te[ds(row0, S), 0:1].rearrange("b one -> one b"))
                    )
                else:
                    trowst = sp.tile([1, S], F32, tag="trowst")
                    nc.sync.dma_start(
                        trowst[:], te[ds(row0, S), 0:1].rearrange("b one -> one b")
                    )
                    nc.vector.tensor_copy(z5[0:1, :], trowst[:])
                tnat = sp.tile([P, 4], F32, tag="tnat")
                nc.sync.dma_start(
                    tnat[:], te[ds(row0, S), 0].rearrange("(c p) -> p c", p=P)
                )

                # ---- transpose inputs into zT chunks ----
                zT = zp.tile([P, 4, S], MMDT, tag="zT")
                for k in range(4):
                    src = tsrc0 if k < 2 else tsrc1
                    fc = k % 2
                    ps = tpp.tile([P, S], MMDT, tag="tps", bufs=1)
                    for c in range(4):
                        src_ap = src[:, c, ds(fc * P, P)]
                        if mode == "f32r":
                            src_ap = _r(src_ap)
                        nc.tensor.transpose(
                            ps[:, ds(c * P, P)], src_ap, ident_m[:]
                        )
                    nc.vector.tensor_copy(zT[:, k, :], ps[:])

                # ---- layer 1 ----
                h1 = hp.tile([P, HC, S], MMDT, tag="hA")
                dh1 = hp.tile([P, HC, S], MMDT, tag="dhA")
                for m in range(HC):
                    psf = mmp.tile([P, S], F32, tag="mmf")
                    for k in range(4):
                        nc.tensor.matmul(
                            psf[:],
                            w1s[:, k, ds(m * P, P)],
                            zT[:, k, :],
                            start=(k == 0),
                            stop=False,
                        )
                    nc.tensor.matmul(
                        psf[:],
                        w15[:, ds(m * P, P)],
                        z5[:],
                        start=False,
                        stop=True,
                    )
                    nc.scalar.activation(
                        h1[:, m, :], psf[:], RELU, bias=b1p[:, m : m + 1]
                    )
                    nc.vector.tensor_scalar(
                        dh1[:, m, :], h1[:, m, :], 0.0, w1rp[:, m : m + 1], GT, MULT
                    )

                # ---- layers 2 and 3 ----
                hprev, dhprev = h1, dh1
                for li, (ws, bp) in enumerate(((w2s, b2p), (w3s, b3p))):
                    hn = hp.tile([P, HC, S], MMDT, tag="hB" if li == 0 else "hA")
                    dhn = hp.tile([P, HC, S], MMDT, tag="dhB" if li == 0 else "dhA")
                    for m in range(HC):
                        psf = mmp.tile([P, S], F32, tag="mmf")
                        pst = mmp.tile([P, S], F32, tag="mmt")
                        for k in range(HC):
                            nc.tensor.matmul(
                                psf[:],
                                ws[:, k, ds(m * P, P)],
                                hprev[:, k, :],
                                start=(k == 0),
                                stop=(k == HC - 1),
                            )
                            nc.tensor.matmul(
                                pst[:],
                                ws[:, k, ds(m * P, P)],
                                dhprev[:, k, :],
                                start=(k == 0),
                                stop=(k == HC - 1),
                            )
                        # relu epilogue on DVE: (psum + b) max 0
                        nc.vector.tensor_scalar(
                            hn[:, m, :], psf[:], bp[:, m : m + 1], 0.0, ADD, MAX
                        )
                        # tangent mask: sign(h) in {0,1} on ACT, then mult on DVE
                        msk = sp.tile([P, S], F32, tag="mask", bufs=1)
                        nc.scalar.activation(msk[:], hn[:, m, :], SIGN)
                        nc.vector.tensor_tensor(dhn[:, m, :], msk[:], pst[:], MULT)
                    hprev, dhprev = hn, dhn

                # ---- layer 4 (no relu) ----
                fnnT = fp.tile([P, DC, S], MMDT, tag="fnnT")
                dfnnT = fp.tile([P, DC, S], MMDT, tag="dfnnT")
                for m in range(DC):
                    psf = mmp.tile([P, S], F32, tag="mmf")
                    pst = mmp.tile([P, S], F32, tag="mmt")
                    for k in range(HC):
                        nc.tensor.matmul(
                            psf[:],
                            w4s[:, k, ds(m * P, P)],
                            hprev[:, k, :],
                            start=(k == 0),
                            stop=(k == HC - 1),
                        )
                        nc.tensor.matmul(
                            pst[:],
                            w4s[:, k, ds(m * P, P)],
                            dhprev[:, k, :],
                            start=(k == 0),
                            stop=(k == HC - 1),
                        )
                    nc.scalar.activation(
                        fnnT[:, m, :], psf[:], IDENT, bias=b4p[:, m : m + 1]
                    )
                    nc.scalar.copy(dfnnT[:, m, :], pst[:])

                # ---- per-stripe t-derived scalars ----
                tsq = sp.tile([P, 4], F32, tag="tsq")
                nc.vector.tensor_tensor(tsq[:], tnat[:], tnat[:], MULT)
                a_ = sp.tile([P, 4], F32, tag="a_")
                nc.vector.tensor_tensor(a_[:], tnat[:], tsq[:], SUB)
                omt = sp.tile([P, 4], F32, tag="omt")
                nc.vector.tensor_scalar(omt[:], tnat[:], -1.0, 1.0, MULT, ADD)
                om2t = sp.tile([P, 4], F32, tag="om2t")
                nc.vector.tensor_scalar(om2t[:], tnat[:], -2.0, 1.0, MULT, ADD)

                # ---- transpose fnn/dfnn back to natural + combine ----
                dt_nat = fp.tile([P, 4, D], F32, tag="dt_nat")
                xt_nat = fp.tile([P, 4, D], F32, tag="xt_nat")
                for cp_i in range(2):
                    psF = tpp.tile([P, 2, D], MMDT, tag="ops", bufs=3)
                    psD = tpp.tile([P, 2, D], MMDT, tag="ops", bufs=3)
                    for ci in range(2):
                        c = 2 * cp_i + ci
                        for fc in range(DC):
                            nc.tensor.transpose(
                                psF[:, ci, ds(fc * P, P)],
                                fnnT[:, fc, ds(c * P, P)],
                                ident_m[:],
                            )
                            nc.tensor.transpose(
                                psD[:, ci, ds(fc * P, P)],
                                dfnnT[:, fc, ds(c * P, P)],
                                ident_m[:],
                            )
                    for ci in range(2):
                        c = 2 * cp_i + ci
                        tc_ = tnat[:, c : c + 1]
                        # dt = x1 - x0 (read x0 before xt ops)
                        nc.vector.tensor_tensor(
                            dt_nat[:, c, :], nat1[:, c, :], nat0[:, c, :], SUB
                        )
                        tm1 = sp.tile([P, D], F32, tag="tmp")
                        nc.scalar.mul(tm1[:], nat1[:, c, :], tc_)  # t*x1
                        nc.vector.tensor_scalar(
                            xt_nat[:, c, :], nat0[:, c, :], omt[:, c : c + 1], None, MULT
                        )
                        tm2 = sp.tile([P, D], F32, tag="tmp")
                        nc.scalar.mul(tm2[:], psF[:, ci, :], a_[:, c : c + 1])
                        nc.vector.tensor_tensor(
                            xt_nat[:, c, :], xt_nat[:, c, :], tm1[:], ADD
                        )
                        nc.vector.tensor_tensor(
                            xt_nat[:, c, :], xt_nat[:, c, :], tm2[:], ADD
                        )
                        tm3 = sp.tile([P, D], F32, tag="tmp")
                        nc.scalar.mul(tm3[:], psF[:, ci, :], om2t[:, c : c + 1])
                        nc.vector.tensor_tensor(
                            dt_nat[:, c, :], dt_nat[:, c, :], tm3[:], ADD
                        )
                        tm4 = sp.tile([P, D], F32, tag="tmp")
                        nc.scalar.mul(tm4[:], psD[:, ci, :], a_[:, c : c + 1])
                        nc.vector.tensor_tensor(
                            dt_nat[:, c, :], dt_nat[:, c, :], tm4[:], ADD
                        )

                nc.sync.dma_start(
                    xte[ds(row0, S), :].rearrange("(c p) f -> p c f", p=P), xt_nat[:]
                )
                nc.sync.dma_start(
                    dte[ds(row0, S), :].rearrange("(c p) f -> p c f", p=P), dt_nat[:]
                )

    nc.compile()
    return nc


def _get_nc(mode=None):
    mode = mode or MODE
    if mode not in _nc_cache:
        _nc_cache[mode] = build(mode)
    return _nc_cache[mode]


def kernel(x0, x1, t, W1, b1, W2, b2, W3, b3, W4, b4, trace=False, **trace_kwargs):
    nc = _get_nc()
    reps = {
        "W1": np.ascontiguousarray(W1, np.float32),
        "b1": np.ascontiguousarray(b1, np.float32),
        "W2": np.ascontiguousarray(W2, np.float32),
        "b2": np.ascontiguousarray(b2, np.float32),
        "W3": np.ascontiguousarray(W3, np.float32),
        "b3": np.ascontiguousarray(b3, np.float32),
        "W4": np.ascontiguousarray(W4, np.float32),
        "b4": np.ascontiguousarray(b4, np.float32),
    }
    in_maps = []
    for c in range(NCORES):
        sl = slice(c * BL, (c + 1) * BL)
        in_maps.append(
            {
                "x0": np.ascontiguousarray(x0[sl], np.float32),
                "x1": np.ascontiguousarray(x1[sl], np.float32),
                "t": np.ascontiguousarray(t[sl], np.float32),
                **reps,
            }
        )
    res = run_bass_kernel_spmd(
        nc, in_maps, list(range(NCORES)), trace=trace, **trace_kwargs
    )
    xt = np.concatenate([res.results[c]["xt"] for c in range(NCORES)], axis=0)
    dt_xt = np.concatenate([res.results[c]["dt_xt"] for c in range(NCORES)], axis=0)
    if trace:
        kernel.last_result = res
    return (xt, dt_xt)


# revision 11
# speedup vs baseline: 1.2237x; 1.2237x over previous
"""Trainium2 Bass kernel for nn_AddInterpolant (dense MLP + JVP interpolant).

Data-parallel over 8 NeuronCores: batch 65536 is split into 8 shards of
8192 rows; the small MLP weights are replicated.  Per core the kernel
computes, for z = concat(x0, x1, t):

    fnn    = W4.(relu(W3.(relu(W2.(relu(W1.z + b1)) + b2)) + b3)) + b4
    dt_fnn = d fnn / dt   (forward-mode JVP with one-hot tangent on t)
    xt     = (1-t) x0 + t x1 + t (1-t) fnn
    dt_xt  = x1 - x0 + (1-2t) fnn + t (1-t) dt_fnn

Layout: activations are kept transposed (features on SBUF partitions,
batch on the free axis) so every layer is a plain accumulation of
128x128 weight-block matmuls; inputs/outputs are transposed on the PE
via identity matmuls.  The t column of z is folded into a zero-padded
K=128 chunk so all matmuls are uniform.  Compute dtype is selectable:
float32r (full-rate fp32 path) or bfloat16.
"""

import os
import sys

for _p in ("/opt/trn_rl_repo",):
    if _p not in sys.path:
        sys.path.insert(0, _p)

import numpy as np

import concourse.mybir as mybir
import concourse.tile as tile
from concourse import bacc
from concourse.bass import ds
from concourse.bass_utils import run_bass_kernel_spmd
from concourse.masks import make_identity

P = 128
D = 256  # state dim
H = 1024  # hidden dim
B = 65536  # global batch
NCORES = 8
BL = B // NCORES  # rows per core
S = 512  # batch columns per stripe
NSTRIPES = BL // S
HC = H // P  # 8 hidden chunks
DC = D // P  # 2 state chunks

F32 = mybir.dt.float32
F32R = mybir.dt.float32r
BF16 = mybir.dt.bfloat16
RELU = mybir.ActivationFunctionType.Relu
IDENT = mybir.ActivationFunctionType.Identity
SIGN = mybir.ActivationFunctionType.Sign
GT = mybir.AluOpType.is_gt
MULT = mybir.AluOpType.mult
ADD = mybir.AluOpType.add
SUB = mybir.AluOpType.subtract
MAX = mybir.AluOpType.max

MODE = os.environ.get("KERNEL_MODE", "bf16")  # "bf16" | "f32r"
LDW_OPT = os.environ.get("KERNEL_LDW_OPT", "0") == "1"

if LDW_OPT:
    import concourse.bass_utils as _bu

    _orig_run_command = _bu.run_command

    def _run_command_ldwopt(cmd, *a, **k):
        cmd = [
            c.replace("--enable-ldw-opt=false", "--enable-ldw-opt=true") for c in cmd
        ]
        return _orig_run_command(cmd, *a, **k)

    _bu.run_command = _run_command_ldwopt

_nc_cache = {}


def _r(ap):
    return ap.bitcast(F32R)


def build(mode=None):
    mode = mode or MODE
    MMDT = BF16 if mode == "bf16" else F32R
    nc = bacc.Bacc(None)

    x0e = nc.declare_dram_parameter("x0", [BL, D], F32, isOutput=False)
    x1e = nc.declare_dram_parameter("x1", [BL, D], F32, isOutput=False)
    te = nc.declare_dram_parameter("t", [BL, 1], F32, isOutput=False)
    W1e = nc.declare_dram_parameter("W1", [2 * D + 1, H], F32, isOutput=False)
    b1e = nc.declare_dram_parameter("b1", [H], F32, isOutput=False)
    W2e = nc.declare_dram_parameter("W2", [H, H], F32, isOutput=False)
    b2e = nc.declare_dram_parameter("b2", [H], F32, isOutput=False)
    W3e = nc.declare_dram_parameter("W3", [H, H], F32, isOutput=False)
    b3e = nc.declare_dram_parameter("b3", [H], F32, isOutput=False)
    W4e = nc.declare_dram_parameter("W4", [H, D], F32, isOutput=False)
    b4e = nc.declare_dram_parameter("b4", [D], F32, isOutput=False)
    xte = nc.declare_dram_parameter("xt", [BL, D], F32, isOutput=True)
    dte = nc.declare_dram_parameter("dt_xt", [BL, D], F32, isOutput=True)

    with tile.TileContext(nc) as tc:
        with (
            tc.tile_pool(name="const", bufs=1) as cp,
            tc.tile_pool(name="z", bufs=1) as zp,
            tc.tile_pool(name="acts", bufs=1) as hp,
            tc.tile_pool(name="outs", bufs=1) as fp,
            tc.tile_pool(name="nat", bufs=2) as npl,
            tc.tile_pool(name="small", bufs=2) as sp,
            tc.tile_pool(name="mm", bufs=2, space="PSUM") as mmp,
            tc.tile_pool(name="tps", bufs=3, space="PSUM") as tpp,
        ):
            # ---- weights in compute dtype ----
            w1s = cp.tile([P, 4, H], MMDT)
            w2s = cp.tile([P, HC, H], MMDT)
            w3s = cp.tile([P, HC, H], MMDT)
            w4s = cp.tile([P, HC, D], MMDT)
            if mode == "f32r":
                # direct DMA via bitcast (verifier accepts f32r-typed DMA)
                nc.sync.dma_start(
                    w1s[:], _r(W1e[0 : 2 * D].rearrange("(o p) n -> p o n", p=P))
                )
                nc.sync.dma_start(w2s[:], _r(W2e.rearrange("(o p) n -> p o n", p=P)))
                nc.sync.dma_start(w3s[:], _r(W3e.rearrange("(o p) n -> p o n", p=P)))
                nc.sync.dma_start(w4s[:], _r(W4e.rearrange("(o p) n -> p o n", p=P)))
            else:
                wst = cp.tile([P, HC, H], F32, name="wstage")
                nc.sync.dma_start(
                    wst[:, 0:4, :], W1e[0 : 2 * D].rearrange("(o p) n -> p o n", p=P)
                )
                nc.vector.tensor_copy(w1s[:], wst[:, 0:4, :])
                nc.sync.dma_start(wst[:], W2e.rearrange("(o p) n -> p o n", p=P))
                nc.vector.tensor_copy(w2s[:], wst[:])
                nc.sync.dma_start(wst[:], W3e.rearrange("(o p) n -> p o n", p=P))
                nc.vector.tensor_copy(w3s[:], wst[:])
                nc.sync.dma_start(
                    wst[:, :, 0:D], W4e.rearrange("(o p) n -> p o n", p=P)
                )
                nc.vector.tensor_copy(w4s[:], wst[:, :, 0:D])
            w1rp = cp.tile([P, HC], F32)
            nc.sync.dma_start(w1rp[:], W1e[2 * D, :].rearrange("(o p) -> p o", p=P))
            b1p = cp.tile([P, HC], F32)
            nc.sync.dma_start(b1p[:], b1e.rearrange("(o p) -> p o", p=P))
            b2p = cp.tile([P, HC], F32)
            nc.sync.dma_start(b2p[:], b2e.rearrange("(o p) -> p o", p=P))
            b3p = cp.tile([P, HC], F32)
            nc.sync.dma_start(b3p[:], b3e.rearrange("(o p) -> p o", p=P))
            b4p = cp.tile([P, DC], F32)
            nc.sync.dma_start(b4p[:], b4e.rearrange("(o p) -> p o", p=P))
            ident = cp.tile([P, P], F32)
            make_identity(nc, ident)
            ident_m = cp.tile([P, P], MMDT)
            nc.vector.tensor_copy(ident_m[:], ident[:])

            # padded "t chunk": Z5 row0 = t (per stripe), rest 0; W15 row0 = W1[512]
            zstage = fp.tile([P, 4, D], F32, tag="dt_nat", name="zstage")
            nc.vector.memset(zstage[:], 0.0)
            z5 = cp.tile([P, S], MMDT)
            nc.vector.tensor_copy(
                z5[:], zstage[:, 0:2, :].rearrange("p a b -> p (a b)")
            )
            w15 = cp.tile([P, H], MMDT)
            nc.vector.tensor_copy(w15[:], zstage[:].rearrange("p a b -> p (a b)"))
            if mode == "f32r":
                nc.sync.dma_start(w15[0:1, :], _r(W1e[2 * D : 2 * D + 1, :]))
            else:
                w15st = sp.tile([1, H], F32, tag="w15st", bufs=1)
                nc.sync.dma_start(w15st[:], W1e[2 * D : 2 * D + 1, :])
                nc.vector.tensor_copy(w15[0:1, :], w15st[:])

            for s in range(NSTRIPES):
                row0 = s * S
                # ---- stripe inputs ----
                nat0 = npl.tile([P, 4, D], F32, tag="nat0")
                nat1 = npl.tile([P, 4, D], F32, tag="nat1")
                if mode == "f32r":
                    nc.sync.dma_start(
                        _r(nat0[:]),
                        _r(x0e[ds(row0, S), :].rearrange("(c p) f -> p c f", p=P)),
                    )
                    nc.sync.dma_start(
                        _r(nat1[:]),
                        _r(x1e[ds(row0, S), :].rearrange("(c p) f -> p c f", p=P)),
                    )
                    tsrc0, tsrc1 = nat0, nat1
                else:
                    nc.sync.dma_start(
                        nat0[:], x0e[ds(row0, S), :].rearrange("(c p) f -> p c f", p=P)
                    )
                    nc.sync.dma_start(
                        nat1[:], x1e[ds(row0, S), :].rearrange("(c p) f -> p c f", p=P)
                    )
                    # convert on idle GpSimd for 1-cyc/row bf16 transposes
                    natb0 = npl.tile([P, 4, D], BF16, tag="natb0")
                    nc.vector.tensor_copy(natb0[:], nat0[:])
                    natb1 = npl.tile([P, 4, D], BF16, tag="natb1")
                    nc.vector.tensor_copy(natb1[:], nat1[:])
                    tsrc0, tsrc1 = natb0, natb1
                if mode == "f32r":
                    nc.sync.dma_start(
                        z5[0:1, :], _r(te[ds(row0, S), 0:1].rearrange("b one -> one b"))
                    )
                else:
                    trowst = sp.tile([1, S], F32, tag="trowst")
                    nc.sync.dma_start(
                        trowst[:], te[ds(row0, S), 0:1].rearrange("b one -> one b")
                    )
                    nc.vector.tensor_copy(z5[0:1, :], trowst[:])
                tnat = sp.tile([P, 4], F32, tag="tnat")
                nc.sync.dma_start(
                    tnat[:], te[ds(row0, S), 0].rearrange("(c p) -> p c", p=P)
                )

                # ---- transpose inputs into zT chunks ----
                zT = zp.tile([P, 4, S], MMDT, tag="zT")
                for k in range(4):
                    src = tsrc0 if k < 2 else tsrc1
                    fc = k % 2
                    ps = tpp.tile([P, S], MMDT, tag="tps", bufs=1)
                    for c in range(4):
                        src_ap = src[:, c, ds(fc * P, P)]
                        if mode == "f32r":
                            src_ap = _r(src_ap)
                        nc.tensor.transpose(
                            ps[:, ds(c * P, P)], src_ap, ident_m[:]
                        )
                    nc.vector.tensor_copy(zT[:, k, :], ps[:])

                # ---- layer 1 ----
                h1 = hp.tile([P, HC, S], MMDT, tag="hA")
                dh1 = hp.tile([P, HC, S], MMDT, tag="dhA")
                for m in range(HC):
                    psf = mmp.tile([P, S], F32, tag="mmf")
                    for k in range(4):
                        nc.tensor.matmul(
                            psf[:],
                            w1s[:, k, ds(m * P, P)],
                            zT[:, k, :],
                            start=(k == 0),
                            stop=False,
                        )
                    nc.tensor.matmul(
                        psf[:],
                        w15[:, ds(m * P, P)],
                        z5[:],
                        start=False,
                        stop=True,
                    )
                    nc.scalar.activation(
                        h1[:, m, :], psf[:], RELU, bias=b1p[:, m : m + 1]
                    )
                    nc.vector.tensor_scalar(
                        dh1[:, m, :], h1[:, m, :], 0.0, w1rp[:, m : m + 1], GT, MULT
                    )

                # ---- layers 2 and 3 ----
                hprev, dhprev = h1, dh1
                for li, (ws, bp) in enumerate(((w2s, b2p), (w3s, b3p))):
                    hn = hp.tile([P, HC, S], MMDT, tag="hB" if li == 0 else "hA")
                    dhn = hp.tile([P, HC, S], MMDT, tag="dhB" if li == 0 else "dhA")
                    for m in range(HC):
                        psf = mmp.tile([P, S], F32, tag="mmf")
                        pst = mmp.tile([P, S], F32, tag="mmt")
                        for k in range(HC):
                            nc.tensor.matmul(
                                psf[:],
                                ws[:, k, ds(m * P, P)],
                                hprev[:, k, :],
                                start=(k == 0),
                                stop=(k == HC - 1),
                            )
                            nc.tensor.matmul(
                                pst[:],
                                ws[:, k, ds(m * P, P)],
                                dhprev[:, k, :],
                                start=(k == 0),
                                stop=(k == HC - 1),
                            )
                        # relu epilogue on DVE: (psum + b) max 0
                        nc.vector.tensor_scalar(
                            hn[:, m, :], psf[:], bp[:, m : m + 1], 0.0, ADD, MAX
                        )
                        # tangent mask: sign(h) in {0,1} on ACT, then mult on DVE
                        msk = sp.tile([P, S], F32, tag="mask", bufs=1)
                        nc.scalar.activation(msk[:], hn[:, m, :], SIGN)
                        nc.vector.tensor_tensor(dhn[:, m, :], msk[:], pst[:], MULT)
                    hprev, dhprev = hn, dhn

                # ---- layer 4 (no relu) ----
                fnnT = fp.tile([P, DC, S], MMDT, tag="fnnT")
                dfnnT = fp.tile([P, DC, S], MMDT, tag="dfnnT")
                for m in range(DC):
                    psf = mmp.tile([P, S], F32, tag="mmf")
                    pst = mmp.tile([P, S], F32, tag="mmt")
                    for k in range(HC):
                        nc.tensor.matmul(
                            psf[:],
                            w4s[:, k, ds(m * P, P)],
                            hprev[:, k, :],
                            start=(k == 0),
                            stop=(k == HC - 1),
                        )
                        nc.tensor.matmul(
                            pst[:],
                            w4s[:, k, ds(m * P, P)],
                            dhprev[:, k, :],
                            start=(k == 0),
                            stop=(k == HC - 1),
                        )
                    nc.scalar.activation(
                        fnnT[:, m, :], psf[:], IDENT, bias=b4p[:, m : m + 1]
                    )
                    nc.scalar.copy(dfnnT[:, m, :], pst[:])

                # ---- per-stripe t-derived scalars ----
                tsq = sp.tile([P, 4], F32, tag="tsq")
                nc.vector.tensor_tensor(tsq[:], tnat[:], tnat[:], MULT)
                a_ = sp.tile([P, 4], F32, tag="a_")
                nc.vector.tensor_tensor(a_[:], tnat[:], tsq[:], SUB)
                omt = sp.tile([P, 4], F32, tag="omt")
                nc.vector.tensor_scalar(omt[:], tnat[:], -1.0, 1.0, MULT, ADD)
                om2t = sp.tile([P, 4], F32, tag="om2t")
                nc.vector.tensor_scalar(om2t[:], tnat[:], -2.0, 1.0, MULT, ADD)

                # ---- transpose fnn/dfnn back to natural + combine ----
                dt_nat = fp.tile([P, 4, D], F32, tag="dt_nat")
                xt_nat = fp.tile([P, 4, D], F32, tag="xt_nat")
                for cp_i in range(2):
                    psF = tpp.tile([P, 2, D], MMDT, tag="ops", bufs=3)
                    psD = tpp.tile([P, 2, D], MMDT, tag="ops", bufs=3)
                    for ci in range(2):
                        c = 2 * cp_i + ci
                        for fc in range(DC):
                            nc.tensor.transpose(
                                psF[:, ci, ds(fc * P, P)],
                                fnnT[:, fc, ds(c * P, P)],
                                ident_m[:],
                            )
                            nc.tensor.transpose(
                                psD[:, ci, ds(fc * P, P)],
                                dfnnT[:, fc, ds(c * P, P)],
                                ident_m[:],
                            )
                    for ci in range(2):
                        c = 2 * cp_i + ci
                        tc_ = tnat[:, c : c + 1]
                        # dt = x1 - x0 (read x0 before xt ops)
                        nc.vector.tensor_tensor(
                            dt_nat[:, c, :], nat1[:, c, :], nat0[:, c, :], SUB
                        )
                        tm1 = sp.tile([P, D], F32, tag="tmp")
                        nc.scalar.mul(tm1[:], nat1[:, c, :], tc_)  # t*x1
                        nc.vector.tensor_scalar(
                            xt_nat[:, c, :], nat0[:, c, :], omt[:, c : c + 1], None, MULT
                        )
                        tm2 = sp.tile([P, D], F32, tag="tmp")
                        nc.scalar.mul(tm2[:], psF[:, ci, :], a_[:, c : c + 1])
                        nc.vector.tensor_tensor(
                            xt_nat[:, c, :], xt_nat[:, c, :], tm1[:], ADD
                        )
                        nc.vector.tensor_tensor(
                            xt_nat[:, c, :], xt_nat[:, c, :], tm2[:], ADD
                        )
                        tm3 = sp.tile([P, D], F32, tag="tmp")
                        nc.scalar.mul(tm3[:], psF[:, ci, :], om2t[:, c : c + 1])
                        nc.vector.tensor_tensor(
                            dt_nat[:, c, :], dt_nat[:, c, :], tm3[:], ADD
                        )
                        tm4 = sp.tile([P, D], F32, tag="tmp")
                        nc.scalar.mul(tm4[:], psD[:, ci, :], a_[:, c : c + 1])
                        nc.vector.tensor_tensor(
                            dt_nat[:, c, :], dt_nat[:, c, :], tm4[:], ADD
                        )

                nc.sync.dma_start(
                    xte[ds(row0, S), :].rearrange("(c p) f -> p c f", p=P), xt_nat[:]
                )
                nc.sync.dma_start(
                    dte[ds(row0, S), :].rearrange("(c p) f -> p c f", p=P), dt_nat[:]
                )

    nc.compile()
    return nc


def _get_nc(mode=None):
    mode = mode or MODE
    if mode not in _nc_cache:
        _nc_cache[mode] = build(mode)
    return _nc_cache[mode]


def kernel(x0, x1, t, W1, b1, W2, b2, W3, b3, W4, b4, trace=False, **trace_kwargs):
    nc = _get_nc()
    reps = {
        "W1": np.ascontiguousarray(W1, np.float32),
        "b1": np.ascontiguousarray(b1, np.float32),
        "W2": np.ascontiguousarray(W2, np.float32),
        "b2": np.ascontiguousarray(b2, np.float32),
        "W3": np.ascontiguousarray(W3, np.float32),
        "b3": np.ascontiguousarray(b3, np.float32),
        "W4": np.ascontiguousarray(W4, np.float32),
        "b4": np.ascontiguousarray(b4, np.float32),
    }
    in_maps = []
    for c in range(NCORES):
        sl = slice(c * BL, (c + 1) * BL)
        in_maps.append(
            {
                "x0": np.ascontiguousarray(x0[sl], np.float32),
                "x1": np.ascontiguousarray(x1[sl], np.float32),
                "t": np.ascontiguousarray(t[sl], np.float32),
                **reps,
            }
        )
    res = run_bass_kernel_spmd(
        nc, in_maps, list(range(NCORES)), trace=trace, **trace_kwargs
    )
    xt = np.concatenate([res.results[c]["xt"] for c in range(NCORES)], axis=0)
    dt_xt = np.concatenate([res.results[c]["dt_xt"] for c in range(NCORES)], axis=0)
    if trace:
        kernel.last_result = res
    return (xt, dt_xt)


# revision 12
# speedup vs baseline: 1.2658x; 1.0344x over previous
"""Trainium2 Bass kernel for nn_AddInterpolant (dense MLP + JVP interpolant).

Data-parallel over 8 NeuronCores: batch 65536 is split into 8 shards of
8192 rows; the small MLP weights are replicated.  Per core the kernel
computes, for z = concat(x0, x1, t):

    fnn    = W4.(relu(W3.(relu(W2.(relu(W1.z + b1)) + b2)) + b3)) + b4
    dt_fnn = d fnn / dt   (forward-mode JVP with one-hot tangent on t)
    xt     = (1-t) x0 + t x1 + t (1-t) fnn
    dt_xt  = x1 - x0 + (1-2t) fnn + t (1-t) dt_fnn

Layout: activations are kept transposed (features on SBUF partitions,
batch on the free axis) so every layer is a plain accumulation of
128x128 weight-block matmuls; inputs/outputs are transposed on the PE
via identity matmuls.  The t column of z is folded into a zero-padded
K=128 chunk so all matmuls are uniform.  Compute dtype is selectable:
float32r (full-rate fp32 path) or bfloat16.
"""

import os
import sys

for _p in ("/opt/trn_rl_repo",):
    if _p not in sys.path:
        sys.path.insert(0, _p)

import numpy as np

import concourse.mybir as mybir
import concourse.tile as tile
from concourse import bacc
from concourse.bass import ds
from concourse.bass_utils import run_bass_kernel_spmd
from concourse.masks import make_identity

P = 128
D = 256  # state dim
H = 1024  # hidden dim
B = 65536  # global batch
NCORES = 8
BL = B // NCORES  # rows per core
S = 512  # batch columns per stripe
NSTRIPES = BL // S
HC = H // P  # 8 hidden chunks
DC = D // P  # 2 state chunks

F32 = mybir.dt.float32
F32R = mybir.dt.float32r
BF16 = mybir.dt.bfloat16
RELU = mybir.ActivationFunctionType.Relu
IDENT = mybir.ActivationFunctionType.Identity
SIGN = mybir.ActivationFunctionType.Sign
GT = mybir.AluOpType.is_gt
MULT = mybir.AluOpType.mult
ADD = mybir.AluOpType.add
SUB = mybir.AluOpType.subtract
MAX = mybir.AluOpType.max

MODE = os.environ.get("KERNEL_MODE", "bf16")  # "bf16" | "f32r"
LDW_OPT = os.environ.get("KERNEL_LDW_OPT", "0") == "1"

if LDW_OPT:
    import concourse.bass_utils as _bu

    _orig_run_command = _bu.run_command

    def _run_command_ldwopt(cmd, *a, **k):
        cmd = [
            c.replace("--enable-ldw-opt=false", "--enable-ldw-opt=true") for c in cmd
        ]
        return _orig_run_command(cmd, *a, **k)

    _bu.run_command = _run_command_ldwopt

_nc_cache = {}


def _r(ap):
    return ap.bitcast(F32R)


def build(mode=None):
    mode = mode or MODE
    MMDT = BF16 if mode == "bf16" else F32R
    nc = bacc.Bacc(None)

    x0e = nc.declare_dram_parameter("x0", [BL, D], F32, isOutput=False)
    x1e = nc.declare_dram_parameter("x1", [BL, D], F32, isOutput=False)
    te = nc.declare_dram_parameter("t", [BL, 1], F32, isOutput=False)
    W1e = nc.declare_dram_parameter("W1", [2 * D + 1, H], F32, isOutput=False)
    b1e = nc.declare_dram_parameter("b1", [H], F32, isOutput=False)
    W2e = nc.declare_dram_parameter("W2", [H, H], F32, isOutput=False)
    b2e = nc.declare_dram_parameter("b2", [H], F32, isOutput=False)
    W3e = nc.declare_dram_parameter("W3", [H, H], F32, isOutput=False)
    b3e = nc.declare_dram_parameter("b3", [H], F32, isOutput=False)
    W4e = nc.declare_dram_parameter("W4", [H, D], F32, isOutput=False)
    b4e = nc.declare_dram_parameter("b4", [D], F32, isOutput=False)
    xte = nc.declare_dram_parameter("xt", [BL, D], F32, isOutput=True)
    dte = nc.declare_dram_parameter("dt_xt", [BL, D], F32, isOutput=True)

    with tile.TileContext(nc) as tc:
        with (
            tc.tile_pool(name="const", bufs=1) as cp,
            tc.tile_pool(name="z", bufs=1) as zp,
            tc.tile_pool(name="acts", bufs=1) as hp,
            tc.tile_pool(name="outs", bufs=1) as fp,
            tc.tile_pool(name="nat", bufs=2) as npl,
            tc.tile_pool(name="small", bufs=2) as sp,
            tc.tile_pool(name="mm", bufs=2, space="PSUM") as mmp,
            tc.tile_pool(name="tps", bufs=3, space="PSUM") as tpp,
        ):
            # ---- weights in compute dtype ----
            w1s = cp.tile([P, 4, H], MMDT)
            w2s = cp.tile([P, HC, H], MMDT)
            w3s = cp.tile([P, HC, H], MMDT)
            w4s = cp.tile([P, HC, D], MMDT)
            if mode == "f32r":
                # direct DMA via bitcast (verifier accepts f32r-typed DMA)
                nc.sync.dma_start(
                    w1s[:], _r(W1e[0 : 2 * D].rearrange("(o p) n -> p o n", p=P))
                )
                nc.sync.dma_start(w2s[:], _r(W2e.rearrange("(o p) n -> p o n", p=P)))
                nc.sync.dma_start(w3s[:], _r(W3e.rearrange("(o p) n -> p o n", p=P)))
                nc.sync.dma_start(w4s[:], _r(W4e.rearrange("(o p) n -> p o n", p=P)))
            else:
                wst = cp.tile([P, HC, H], F32, name="wstage")
                nc.sync.dma_start(
                    wst[:, 0:4, :], W1e[0 : 2 * D].rearrange("(o p) n -> p o n", p=P)
                )
                nc.vector.tensor_copy(w1s[:], wst[:, 0:4, :])
                nc.sync.dma_start(wst[:], W2e.rearrange("(o p) n -> p o n", p=P))
                nc.vector.tensor_copy(w2s[:], wst[:])
                nc.sync.dma_start(wst[:], W3e.rearrange("(o p) n -> p o n", p=P))
                nc.vector.tensor_copy(w3s[:], wst[:])
                nc.sync.dma_start(
                    wst[:, :, 0:D], W4e.rearrange("(o p) n -> p o n", p=P)
                )
                nc.vector.tensor_copy(w4s[:], wst[:, :, 0:D])
            w1rp = cp.tile([P, HC], F32)
            nc.sync.dma_start(w1rp[:], W1e[2 * D, :].rearrange("(o p) -> p o", p=P))
            b1p = cp.tile([P, HC], F32)
            nc.sync.dma_start(b1p[:], b1e.rearrange("(o p) -> p o", p=P))
            b2p = cp.tile([P, HC], F32)
            nc.sync.dma_start(b2p[:], b2e.rearrange("(o p) -> p o", p=P))
            b3p = cp.tile([P, HC], F32)
            nc.sync.dma_start(b3p[:], b3e.rearrange("(o p) -> p o", p=P))
            b4p = cp.tile([P, DC], F32)
            nc.sync.dma_start(b4p[:], b4e.rearrange("(o p) -> p o", p=P))
            ident = cp.tile([P, P], F32)
            make_identity(nc, ident)
            ident_m = cp.tile([P, P], MMDT)
            nc.vector.tensor_copy(ident_m[:], ident[:])

            # padded "t chunk": Z5 row0 = t (per stripe), rest 0; W15 row0 = W1[512]
            zstage = fp.tile([P, 4, D], F32, tag="dt_nat", name="zstage")
            nc.vector.memset(zstage[:], 0.0)
            z5 = cp.tile([P, S], MMDT)
            nc.vector.tensor_copy(
                z5[:], zstage[:, 0:2, :].rearrange("p a b -> p (a b)")
            )
            w15 = cp.tile([P, H], MMDT)
            nc.vector.tensor_copy(w15[:], zstage[:].rearrange("p a b -> p (a b)"))
            if mode == "f32r":
                nc.sync.dma_start(w15[0:1, :], _r(W1e[2 * D : 2 * D + 1, :]))
            else:
                w15st = sp.tile([1, H], F32, tag="w15st", bufs=1)
                nc.sync.dma_start(w15st[:], W1e[2 * D : 2 * D + 1, :])
                nc.vector.tensor_copy(w15[0:1, :], w15st[:])

            def emit_input(s):
                """DMA + cast the stripe inputs; returns tiles for later stages."""
                row0 = s * S
                nat0 = npl.tile([P, 4, D], F32, tag="nat0", name=f"nat0_{s}")
                nat1 = npl.tile([P, 4, D], F32, tag="nat1", name=f"nat1_{s}")
                if mode == "f32r":
                    nc.sync.dma_start(
                        _r(nat0[:]),
                        _r(x0e[ds(row0, S), :].rearrange("(c p) f -> p c f", p=P)),
                    )
                    nc.sync.dma_start(
                        _r(nat1[:]),
                        _r(x1e[ds(row0, S), :].rearrange("(c p) f -> p c f", p=P)),
                    )
                    tsrc0, tsrc1 = nat0, nat1
                else:
                    nc.sync.dma_start(
                        nat0[:], x0e[ds(row0, S), :].rearrange("(c p) f -> p c f", p=P)
                    )
                    nc.sync.dma_start(
                        nat1[:], x1e[ds(row0, S), :].rearrange("(c p) f -> p c f", p=P)
                    )
                    natb0 = npl.tile([P, 4, D], BF16, tag="natb0", name=f"natb0_{s}")
                    nc.vector.tensor_copy(natb0[:], nat0[:])
                    natb1 = npl.tile([P, 4, D], BF16, tag="natb1", name=f"natb1_{s}")
                    nc.vector.tensor_copy(natb1[:], nat1[:])
                    tsrc0, tsrc1 = natb0, natb1
                tnat = sp.tile([P, 4], F32, tag="tnat", name=f"tnat_{s}")
                nc.sync.dma_start(
                    tnat[:], te[ds(row0, S), 0].rearrange("(c p) -> p c", p=P)
                )
                return nat0, nat1, tsrc0, tsrc1, tnat

            def emit_trow(s):
                row0 = s * S
                if mode == "f32r":
                    nc.sync.dma_start(
                        z5[0:1, :], _r(te[ds(row0, S), 0:1].rearrange("b one -> one b"))
                    )
                else:
                    trowst = sp.tile([1, S], F32, tag="trowst", name=f"trowst_{s}")
                    nc.sync.dma_start(
                        trowst[:], te[ds(row0, S), 0:1].rearrange("b one -> one b")
                    )
                    nc.vector.tensor_copy(z5[0:1, :], trowst[:])

            pending = emit_input(0)
            for s in range(NSTRIPES):
                row0 = s * S
                nat0, nat1, tsrc0, tsrc1, tnat = pending
                emit_trow(s)

                # ---- transpose inputs into zT chunks ----
                zT = zp.tile([P, 4, S], MMDT, tag="zT")
                for k in range(4):
                    src = tsrc0 if k < 2 else tsrc1
                    fc = k % 2
                    ps = tpp.tile([P, S], MMDT, tag="tps", bufs=2)
                    for c in range(4):
                        src_ap = src[:, c, ds(fc * P, P)]
                        if mode == "f32r":
                            src_ap = _r(src_ap)
                        nc.tensor.transpose(
                            ps[:, ds(c * P, P)], src_ap, ident_m[:]
                        )
                    nc.vector.tensor_copy(zT[:, k, :], ps[:])

                # ---- layer 1 ----
                h1 = hp.tile([P, HC, S], MMDT, tag="hA")
                dh1 = hp.tile([P, HC, S], MMDT, tag="dhA")
                for m in range(HC):
                    psf = mmp.tile([P, S], F32, tag="mmf")
                    for k in range(4):
                        nc.tensor.matmul(
                            psf[:],
                            w1s[:, k, ds(m * P, P)],
                            zT[:, k, :],
                            start=(k == 0),
                            stop=False,
                        )
                    nc.tensor.matmul(
                        psf[:],
                        w15[:, ds(m * P, P)],
                        z5[:],
                        start=False,
                        stop=True,
                    )
                    nc.scalar.activation(
                        h1[:, m, :], psf[:], RELU, bias=b1p[:, m : m + 1]
                    )
                    nc.vector.tensor_scalar(
                        dh1[:, m, :], h1[:, m, :], 0.0, w1rp[:, m : m + 1], GT, MULT
                    )

                # ---- layers 2 and 3 ----
                hprev, dhprev = h1, dh1
                for li, (ws, bp) in enumerate(((w2s, b2p), (w3s, b3p))):
                    hn = hp.tile([P, HC, S], MMDT, tag="hB" if li == 0 else "hA")
                    dhn = hp.tile([P, HC, S], MMDT, tag="dhB" if li == 0 else "dhA")
                    for m in range(HC):
                        psf = mmp.tile([P, S], F32, tag="mmf")
                        pst = mmp.tile([P, S], F32, tag="mmt", bufs=4)
                        for k in range(HC):
                            nc.tensor.matmul(
                                psf[:],
                                ws[:, k, ds(m * P, P)],
                                hprev[:, k, :],
                                start=(k == 0),
                                stop=(k == HC - 1),
                            )
                            nc.tensor.matmul(
                                pst[:],
                                ws[:, k, ds(m * P, P)],
                                dhprev[:, k, :],
                                start=(k == 0),
                                stop=(k == HC - 1),
                            )
                        # relu epilogue on DVE: (psum + b) max 0
                        nc.vector.tensor_scalar(
                            hn[:, m, :], psf[:], bp[:, m : m + 1], 0.0, ADD, MAX
                        )
                        # tangent mask: sign(h) in {0,1} on ACT, then mult on DVE
                        msk = sp.tile([P, S], F32, tag="mask", bufs=1)
                        nc.scalar.activation(msk[:], hn[:, m, :], SIGN)
                        nc.vector.tensor_tensor(dhn[:, m, :], msk[:], pst[:], MULT)
                    hprev, dhprev = hn, dhn

                # ---- layer 4 (no relu) ----
                fnnT = fp.tile([P, DC, S], MMDT, tag="fnnT")
                dfnnT = fp.tile([P, DC, S], MMDT, tag="dfnnT")
                for m in range(DC):
                    psf = mmp.tile([P, S], F32, tag="mmf")
                    pst = mmp.tile([P, S], F32, tag="mmt", bufs=4)
                    for k in range(HC):
                        nc.tensor.matmul(
                            psf[:],
                            w4s[:, k, ds(m * P, P)],
                            hprev[:, k, :],
                            start=(k == 0),
                            stop=(k == HC - 1),
                        )
                        nc.tensor.matmul(
                            pst[:],
                            w4s[:, k, ds(m * P, P)],
                            dhprev[:, k, :],
                            start=(k == 0),
                            stop=(k == HC - 1),
                        )
                    nc.scalar.activation(
                        fnnT[:, m, :], psf[:], IDENT, bias=b4p[:, m : m + 1]
                    )
                    nc.scalar.copy(dfnnT[:, m, :], pst[:])

                if s + 1 < NSTRIPES:
                    pending = emit_input(s + 1)

                # ---- per-stripe t-derived scalars ----
                tsq = sp.tile([P, 4], F32, tag="tsq")
                nc.vector.tensor_tensor(tsq[:], tnat[:], tnat[:], MULT)
                a_ = sp.tile([P, 4], F32, tag="a_")
                nc.vector.tensor_tensor(a_[:], tnat[:], tsq[:], SUB)
                omt = sp.tile([P, 4], F32, tag="omt")
                nc.vector.tensor_scalar(omt[:], tnat[:], -1.0, 1.0, MULT, ADD)
                om2t = sp.tile([P, 4], F32, tag="om2t")
                nc.vector.tensor_scalar(om2t[:], tnat[:], -2.0, 1.0, MULT, ADD)

                # ---- transpose fnn/dfnn back to natural + combine ----
                dt_nat = fp.tile([P, 4, D], F32, tag="dt_nat")
                xt_nat = fp.tile([P, 4, D], F32, tag="xt_nat")
                for cp_i in range(2):
                    psF = mmp.tile([P, 2, D], MMDT, tag="mmt", bufs=4)
                    psD = mmp.tile([P, 2, D], MMDT, tag="mmt", bufs=4)
                    for ci in range(2):
                        c = 2 * cp_i + ci
                        for fc in range(DC):
                            nc.tensor.transpose(
                                psF[:, ci, ds(fc * P, P)],
                                fnnT[:, fc, ds(c * P, P)],
                                ident_m[:],
                            )
                            nc.tensor.transpose(
                                psD[:, ci, ds(fc * P, P)],
                                dfnnT[:, fc, ds(c * P, P)],
                                ident_m[:],
                            )
                    for ci in range(2):
                        c = 2 * cp_i + ci
                        tc_ = tnat[:, c : c + 1]
                        # dt = x1 - x0 (read x0 before xt ops)
                        nc.vector.tensor_tensor(
                            dt_nat[:, c, :], nat1[:, c, :], nat0[:, c, :], SUB
                        )
                        tm1 = sp.tile([P, D], F32, tag="tmp")
                        nc.scalar.mul(tm1[:], nat1[:, c, :], tc_)  # t*x1
                        nc.vector.tensor_scalar(
                            xt_nat[:, c, :], nat0[:, c, :], omt[:, c : c + 1], None, MULT
                        )
                        tm2 = sp.tile([P, D], F32, tag="tmp")
                        nc.scalar.mul(tm2[:], psF[:, ci, :], a_[:, c : c + 1])
                        nc.vector.tensor_tensor(
                            xt_nat[:, c, :], xt_nat[:, c, :], tm1[:], ADD
                        )
                        nc.vector.tensor_tensor(
                            xt_nat[:, c, :], xt_nat[:, c, :], tm2[:], ADD
                        )
                        tm3 = sp.tile([P, D], F32, tag="tmp")
                        nc.scalar.mul(tm3[:], psF[:, ci, :], om2t[:, c : c + 1])
                        nc.vector.tensor_tensor(
                            dt_nat[:, c, :], dt_nat[:, c, :], tm3[:], ADD
                        )
                        tm4 = sp.tile([P, D], F32, tag="tmp")
                        nc.scalar.mul(tm4[:], psD[:, ci, :], a_[:, c : c + 1])
                        nc.vector.tensor_tensor(
                            dt_nat[:, c, :], dt_nat[:, c, :], tm4[:], ADD
                        )

                nc.sync.dma_start(
                    xte[ds(row0, S), :].rearrange("(c p) f -> p c f", p=P), xt_nat[:]
                )
                nc.sync.dma_start(
                    dte[ds(row0, S), :].rearrange("(c p) f -> p c f", p=P), dt_nat[:]
                )

    nc.compile()
    return nc


def _get_nc(mode=None):
    mode = mode or MODE
    if mode not in _nc_cache:
        _nc_cache[mode] = build(mode)
    return _nc_cache[mode]


def kernel(x0, x1, t, W1, b1, W2, b2, W3, b3, W4, b4, trace=False, **trace_kwargs):
    nc = _get_nc()
    reps = {
        "W1": np.ascontiguousarray(W1, np.float32),
        "b1": np.ascontiguousarray(b1, np.float32),
        "W2": np.ascontiguousarray(W2, np.float32),
        "b2": np.ascontiguousarray(b2, np.float32),
        "W3": np.ascontiguousarray(W3, np.float32),
        "b3": np.ascontiguousarray(b3, np.float32),
        "W4": np.ascontiguousarray(W4, np.float32),
        "b4": np.ascontiguousarray(b4, np.float32),
    }
    in_maps = []
    for c in range(NCORES):
        sl = slice(c * BL, (c + 1) * BL)
        in_maps.append(
            {
                "x0": np.ascontiguousarray(x0[sl], np.float32),
                "x1": np.ascontiguousarray(x1[sl], np.float32),
                "t": np.ascontiguousarray(t[sl], np.float32),
                **reps,
            }
        )
    res = run_bass_kernel_spmd(
        nc, in_maps, list(range(NCORES)), trace=trace, **trace_kwargs
    )
    xt = np.concatenate([res.results[c]["xt"] for c in range(NCORES)], axis=0)
    dt_xt = np.concatenate([res.results[c]["dt_xt"] for c in range(NCORES)], axis=0)
    if trace:
        kernel.last_result = res
    return (xt, dt_xt)
